# revision 15
# baseline (speedup 1.0000x reference)
"""Trainium2 Bass kernel for per-sample covariance pooling + fc + L2 norm.

Reference computation (per sample b of B=32):
    xc  = x[b] - mean(x[b], axis=0)            # x[b]: [N=20000, D=64]
    cov = xc.T @ xc / (N-1)                    # [64, 64]
    out = normalize(cov.flatten() @ W + b)     # [256]

Kernel formulation (scale/norm invariant):
    G = x.T @ x, s = sum(x, axis=0)            # one PE pass over x
    cov = (G - s s^T / N) / (N-1)
    out = normalize(cov.flatten() @ W + b)

Sharding: data-parallel over batch, 4 samples per core on 8 cores.
W and bias are replicated. x is pre-packed on the host into fp8 e4m3
(rel err vs f64 reference ~2.3e-3, gate is 2e-2) in the DoubleRow
matmul layout: blocks of 256 rows split into two 128-row k-tiles that
one fp8 matmul instruction contracts in a single pass (2x PE rate).
A baked-in ones column gives s for free as row 64 of G. W/feat are
fp16. All x DMAs are issued before the W DMAs on both HWDGE rings, so
the fc matmuls (which need feats of all 4 samples, i.e. the entire x
stream) hide completely under the W stream instead of trailing it.
"""

import sys

import numpy as np
import ml_dtypes

for _p in ("/opt/trn_rl_repo",):
    if _p not in sys.path:
        sys.path.append(_p)

# Problem shapes (hardcoded per contract).
B, N, D, OUT = 32, 20000, 64, 256
NCORES = 8
BPC = B // NCORES            # samples per core
P = 128                      # SBUF partitions
AUG = D + 1                  # x columns + ones column
BLK = 2 * P                  # rows contracted per DoubleRow matmul
NB = (N + BLK - 1) // BLK    # 79 blocks of 256 rows
NPAD = NB * BLK              # 20224 rows after zero padding
FB = 2 * AUG                 # free bytes per partition per block (two k-tiles)
KC = (D * D) // P            # 32 fc contraction chunks
WSLICES = 8                  # W DMA slices (each covers 4 fc chunks)
# x DMA schedule per sample: (block offset, blocks). Last tile small so
# the final G chunks finish right after the stream ends.
DMA_TILES = [(0, 27), (27, 26), (53, 20), (73, 6)]
FILL_PER_TILE = 1            # HAM-warming dummy matmuls per x tile

_CACHE = {}


def _split_drain_and_barrier(self, tick_clock, wait_clock):
    """Replacement for TileContext._drain_and_barrier emitting one drain per
    sem wait: this walrus vintage rejects >1 sync-wait per instruction."""
    import bass_rust
    import concourse.mybir as mybir

    drain_bi = self.nc.sync.drain()
    inst = drain_bi.ins
    wait_clock.add_sem_waits(
        drain_bi.ins, bass_rust.ScopedClock({None: tick_clock.global_clock})
    )
    waits = list(inst.sync_info.on_wait) if inst.sync_info else []
    if len(waits) > 1:
        # one pure sem-wait NoOp per extra wait (cheaper than extra drains)
        inst.sync_info = mybir.SyncInfo(on_wait=waits[:1], on_update=[])
        for w in waits[1:]:
            nop = mybir.InstNoOp(
                name=f"tailwait-{w.ant_name}",
                engine=mybir.EngineType.SP,
                sync_info=mybir.SyncInfo(on_wait=[w], on_update=[]),
                bass_nofuse=True,
            )
            self.nc.sync.add_instruction(nop)

    self.nc.all_engine_barrier()
    assert self.sems is not None
    popped = self.nc._tile_sem_poison_stack.pop()
    assert popped is self._sem_poison
    self.nc.clear_and_free_semaphores(list(self.sems.allocated().values()))
    self.nc.all_engine_barrier()


def _build_nc():
    import types

    import concourse.bass as bass
    import concourse.mybir as mybir
    from concourse.tile import TileContext

    dt = mybir.dt
    AF = mybir.ActivationFunctionType
    PM = mybir.MatmulPerfMode
    nc = bass.Bass()

    xin = nc.dram_tensor("xin", [BPC, NB * FB * P], dt.float8e4, kind="ExternalInput")
    win = nc.dram_tensor("win", [P, KC * OUT], dt.float16, kind="ExternalInput")
    # cols 0:OUT: bias; cols OUT:OUT+BPC: ones (same row -- matmul
    # operands must start at partition 0/32/64)
    bin_ = nc.dram_tensor("bin", [1, OUT + BPC], dt.float32, kind="ExternalInput")
    yout = nc.dram_tensor("yout", [BPC, OUT], dt.float32, kind="ExternalOutput")

    # Walrus single-sync-wait discipline (see _split_drain_and_barrier):
    #  - x tiles get one pool slot per DMA (no slot reuse -> DMAs need 0 waits)
    #  - per-sample psum G tiles are not reused (gpsum bufs=BPC)
    #  - the s row is read/scaled on a single engine (DVE); cross-engine
    #    joins are relayed so same-engine waits merge
    #  - PE "observes" each W slice's DMA lane via a dummy matmul right
    #    before the first fc matmul that reads the slice.
    tc = TileContext(nc)
    tc._drain_and_barrier = types.MethodType(_split_drain_and_barrier, tc)
    with tc:
        with (
            tc.tile_pool(name="const", bufs=1) as cpool,
            tc.tile_pool(name="xp", bufs=len(DMA_TILES) * BPC) as xpool,
            tc.tile_pool(name="small", bufs=2) as spool,
            tc.tile_pool(name="featp", bufs=1) as fpool,
            tc.tile_pool(name="gpsum", bufs=BPC, space="PSUM") as gpool,
            tc.tile_pool(name="rpsum", bufs=1, space="PSUM") as rpool,
            tc.tile_pool(name="opsum", bufs=1, space="PSUM") as opool,
        ):
            w_sb = cpool.tile([P, KC * OUT], dt.float16)
            bias_sb = cpool.tile([1, OUT + BPC], dt.float32)
            nc.scalar.dma_start(out=bias_sb[:], in_=bin_[:])

            # The two HWDGE rings drain strictly in issue order per ring.
            ring = [nc.sync, nc.scalar]
            rr = [0]

            def ring_dma(out, in_, force=None):
                r = force if force is not None else rr[0] % 2
                if force is None:
                    rr[0] += 1
                ring[r].dma_start(out=out, in_=in_)

            # feat_sb[p, c, bb] = flattened cov for sample bb, fc-chunk
            # layout: element k = c*128 + p of cov.flatten(); chunk c stacks
            # cov[:, 2c] on partitions 0:64 and cov[:, 2c+1] on 64:128.
            feat_sb = fpool.tile([P, KC, BPC], dt.float16)

            po = opool.tile([BPC, OUT], dt.float32)
            pdum = opool.tile([1, 512], dt.float32, tag="pdum")

            # Pre-warm the PE clock gate (HAM) with dummy matmuls on a memset
            # tile while the first x tile is still in flight: the gate needs
            # ~3.4 us of sustained activity to lift the 1.2 GHz cold throttle.
            dumsrc = cpool.tile([P, 512], dt.float8e4)
            nc.vector.memset(dumsrc[:], 0.5)
            for _ in range(8):
                nc.tensor.matmul(
                    pdum[:], lhsT=dumsrc[:, 0:1], rhs=dumsrc[:, 0:512],
                    start=True, stop=True,
                )

            # Scratch for the s-column -> s-row transpose. Only column 0 is
            # ever written; the 32x32 block transpose routes in-column j to
            # out-row j, so the junk in columns 1:32 lands only on output
            # rows we never read (everything but rows 0 and 32).
            s32 = cpool.tile([D, 32], dt.float32, tag="s32")

            # One psum bank holds all four samples' (rps, sps) regions at
            # disjoint column offsets: each region is only touched by its own
            # accumulation chain, so interleaved start=True zero-marking of
            # the shared bank never clobbers live data.
            rq = rpool.tile([D, 4 * P], dt.float32, tag="rq")

            def do_sample(bb):
                pg = gpool.tile([D, D], dt.float32, tag="pg")
                sps = rq[:, bb * P + D : bb * P + D + 1]
                xts = []
                for ti, (i0, nblk) in enumerate(DMA_TILES):
                    xt = xpool.tile([P, nblk * FB], dt.float8e4, tag="xt")
                    xts.append(xt)
                    ring_dma(
                        xt[:],
                        xin[bb, i0 * FB * P : (i0 + nblk) * FB * P].rearrange(
                            "(p f) -> p f", p=P
                        ),
                    )
                for ti, (i0, nblk) in enumerate(DMA_TILES):
                    xt = xts[ti]
                    for j in range(nblk):
                        # Block layout: [x_t0(64) | x_t1(64) | one_t0 | one_t1]
                        # per partition. DoubleRow needs the two k-tiles of
                        # stationary weights contiguous, so the ones bytes sit
                        # after the x pair and feed a second 1-column matmul
                        # that accumulates s into its own psum.
                        chx = xt[:, j * FB : j * FB + 2 * D].rearrange(
                            "p (two f) -> p two f", two=2
                        )
                        ch1 = xt[:, j * FB + 2 * D : (j + 1) * FB].rearrange(
                            "p (two f) -> p two f", two=2
                        )
                        nc.tensor.matmul(
                            pg[:],
                            lhsT=chx,
                            rhs=chx,
                            start=(i0 + j == 0),
                            stop=(i0 + j == NB - 1),
                            perf_mode=PM.DoubleRow,
                        )
                        nc.tensor.matmul(
                            sps,
                            lhsT=chx,
                            rhs=ch1,
                            start=(i0 + j == 0),
                            stop=(i0 + j == NB - 1),
                            perf_mode=PM.DoubleRow,
                            skip_group_check=True,
                        )
                    # HAM-warming filler: keeps the PE activity monitor from
                    # re-throttling the clock to 1.2 GHz during DMA stalls.
                    for _ in range(FILL_PER_TILE):
                        nc.tensor.matmul(
                            pdum[:, 0:256], lhsT=xt[:, 0:1], rhs=xt[:, 0:256],
                            start=True, stop=True,
                        )
                # pg = G [64,64], sps = s [64,1] as a column. Rebuild s as a
                # row via a DVE 32x32 block transpose (col 0 of s32 -> rows 0
                # and 32 of sT), stitch the halves, then R = (s/(N(N-1))) s^T
                # into its own psum, relay-copied to SBUF, and feat =
                # G/(N-1) - R fused into the (strided) feat_sb copies.
                # Cross-engine joins funnel through DVE ticks so each
                # instruction needs at most one sync wait.
                nc.vector.tensor_copy(s32[:, 0:1], sps)
                sT = spool.tile([D, 32], dt.float32, tag="sT")
                nc.vector.transpose(sT[:], s32[:])
                s_pos = spool.tile([1, D], dt.float32, tag="spos")
                nc.vector.tensor_copy(s_pos[0:1, 0:32], sT[0:1, 0:32])
                nc.vector.tensor_copy(s_pos[0:1, 32:D], sT[32:33, 0:32])
                s_scl = spool.tile([1, D], dt.float32, tag="sscl")
                nc.vector.tensor_scalar_mul(
                    s_scl[:], s_pos[:], 1.0 / (N * (N - 1.0))
                )
                rps = rq[:, bb * P : bb * P + D]
                nc.tensor.matmul(
                    rps, lhsT=s_scl[:], rhs=s_pos[:], start=True, stop=True,
                    skip_group_check=True,
                )
                rsb = spool.tile([D, D], dt.float32, tag="rsb")
                nc.vector.tensor_copy(rsb[:], rps)
                ge = pg[0:D, 0:D].rearrange("p (c two) -> p c two", two=2)
                re = rsb[:].rearrange("p (c two) -> p c two", two=2)
                # feat = G/(N-1) - s s^T/(N(N-1))  (= cov), cast to fp16
                nc.vector.scalar_tensor_tensor(
                    feat_sb[0:D, :, bb], ge[:, :, 0], 1.0 / (N - 1.0),
                    re[:, :, 0], op0=mybir.AluOpType.mult,
                    op1=mybir.AluOpType.subtract,
                )
                nc.vector.scalar_tensor_tensor(
                    feat_sb[D:P, :, bb], ge[:, :, 1], 1.0 / (N - 1.0),
                    re[:, :, 1], op0=mybir.AluOpType.mult,
                    op1=mybir.AluOpType.subtract,
                )
                # keep the PE array warm across the sample-boundary stall
                for _ in range(0 if bb == 0 else 2):
                    nc.tensor.matmul(
                        pdum[:], lhsT=xts[-1][:, 0:1], rhs=xts[-1][:, 0:512],
                        start=True, stop=True,
                    )

            for bb in range(BPC):
                do_sample(bb)

            # W rides both rings AFTER the entire x stream: the fc matmuls
            # pace behind the arriving W slices and hide under their DMA
            # time instead of trailing the x stream.
            WSL = KC * OUT // WSLICES
            for c in range(WSLICES):
                ring_dma(
                    w_sb[:, c * WSL : (c + 1) * WSL],
                    win[:, c * WSL : (c + 1) * WSL],
                    force=c % 2,
                )

            # Open the fc accumulation with the bias row: po = 1 * bias'.
            nc.tensor.matmul(
                po[:], lhsT=bias_sb[0:1, OUT : OUT + BPC], rhs=bias_sb[0:1, 0:OUT],
                start=True, stop=False,
            )
            # fc: out[bb, o] = bias'[o] + sum_k feat[k, bb] * W[k, o].
            # Before the first chunk of each W slice, a 1x1 dummy matmul
            # observes that slice's DMA lane so the fc matmul itself only
            # needs its feat (DVE) wait.
            CPS = KC // WSLICES
            for c in range(KC):
                if c % CPS == 0:
                    sl = c // CPS
                    nc.tensor.matmul(
                        pdum[0:1, 0:1],
                        lhsT=w_sb[0:1, sl * WSL : sl * WSL + 1],
                        rhs=w_sb[0:1, sl * WSL : sl * WSL + 1],
                        start=True, stop=True,
                    )
                nc.tensor.matmul(
                    po[:],
                    lhsT=feat_sb[:, c, :],
                    rhs=w_sb[:, c * OUT : (c + 1) * OUT],
                    start=False,
                    stop=(c == KC - 1),
                )

            # L2 normalize rows: out = po / sqrt(sum(po^2)). ACT Square with
            # row-sum accumulator (square and sqrt share one ACT table set),
            # ACT sqrt, DVE reciprocal, one DVE scale.
            sq = spool.tile([BPC, OUT], dt.float32, tag="sq")
            ss = spool.tile([BPC, 1], dt.float32, tag="ss")
            nc.scalar.activation(sq[:], po[:], AF.Square, accum_out=ss[:])
            nrm = spool.tile([BPC, 1], dt.float32, tag="nrm")
            nc.scalar.activation(nrm[:], ss[:], AF.Sqrt)
            inv = spool.tile([BPC, 1], dt.float32, tag="inv")
            nc.vector.reciprocal(inv[:], nrm[:])
            out_sb = spool.tile([BPC, OUT], dt.float32, tag="osb")
            nc.vector.tensor_scalar_mul(out_sb[:], po[:], inv[:])
            # SWDGE: an HWDGE yout DMA would need a DMAHW lane-reuse wait on
            # top of its DVE data wait (2 waits > walrus limit).
            nc.gpsimd.dma_start(out=yout[:], in_=out_sb[:])

    return nc


def _get_nc():
    if "nc" not in _CACHE:
        _CACHE["nc"] = _build_nc()
    return _CACHE["nc"]


def _pack_inputs(x, W, b):
    x = np.asarray(x, dtype=np.float32)
    W = np.asarray(W, dtype=np.float32)
    b = np.asarray(b, dtype=np.float32)

    xpad = np.zeros((B, NPAD, D), dtype=ml_dtypes.float8_e4m3)
    xpad[:, :N, :] = x.astype(ml_dtypes.float8_e4m3)
    # row n = block i*256 + ktile t*128 + partition p. Per-partition block
    # layout: [x_t0(64) | x_t1(64) | one_t0 | one_t1] (130 B); DMA tiles are
    # regrouped so each dma_start reads one contiguous DRAM extent.
    xT = xpad.reshape(B, NB, 2, P, D).transpose(0, 3, 1, 2, 4)  # [B,P,NB,2,D]
    xT = xT.reshape(B, P, NB, 2 * D)
    ones = np.ones((B, P, NB, 2), dtype=ml_dtypes.float8_e4m3)
    augT = np.concatenate([xT, ones], axis=3).reshape(B, P, NB * FB)
    parts = []
    for (i0, nblk) in DMA_TILES:
        blk = augT[:, :, i0 * FB : (i0 + nblk) * FB]
        parts.append(blk.reshape(B, P * nblk * FB))
    augT = np.ascontiguousarray(np.concatenate(parts, axis=1))

    wp = np.ascontiguousarray(
        W.reshape(KC, P, OUT).transpose(1, 0, 2)
    ).reshape(P, KC * OUT).astype(np.float16)
    bp = np.concatenate([b, np.ones(BPC, np.float32)]).reshape(1, OUT + BPC)

    return [
        {
            "xin": np.ascontiguousarray(augT[c * BPC : (c + 1) * BPC]),
            "win": wp,
            "bin": bp,
        }
        for c in range(NCORES)
    ]


def run(x, W, b, trace=False):
    from concourse.bass_utils import run_bass_kernel_spmd

    nc = _get_nc()
    in_maps = _pack_inputs(x, W, b)
    res = run_bass_kernel_spmd(nc, in_maps, list(range(NCORES)), trace=trace)
    out = np.concatenate(
        [res.results[c]["yout"] for c in range(NCORES)], axis=0
    ).astype(np.float32)
    return out, res


def kernel(x, W, b):
    out, _ = run(x, W, b, trace=False)
    return out


# revision 17
# speedup vs baseline: 1.5222x; 1.5222x over previous
"""Trainium2 Bass kernel for per-sample covariance pooling + fc + L2 norm.

Reference computation (per sample b of B=32):
    xc  = x[b] - mean(x[b], axis=0)            # x[b]: [N=20000, D=64]
    cov = xc.T @ xc / (N-1)                    # [64, 64]
    out = normalize(cov.flatten() @ W + b)     # [256]

Kernel formulation (scale/norm invariant):
    G = x.T @ x, s = sum(x, axis=0)            # one PE pass over x
    cov = (G - s s^T / N) / (N-1)
    out = normalize(cov.flatten() @ W + b)

Sharding: data-parallel over batch, 4 samples per core on 8 cores; W
and bias replicated. x is host-packed to fp8 e4m3 (end-to-end rel err
~2.3e-3 vs the 2e-2 gate). Two samples ride side by side per
partition row: chunk layout [x_a(64) | x_b(64) | ones(1)], so the
Gram matmul has a 128-column stationary operand -- exactly the shape
that triggers the compiler's Fast Weight Load (4 fp8/cycle; the
dominant LDWEIGHTS cost of tall-skinny Grams drops 4x) -- and one
matmul per 128 rows yields both samples' G blocks plus both column
sums (from the ones column) in a [128, 129] psum. DoubleRow mode is
deliberately NOT used: at free-dim 64 it disables FWL and measures
~3x slower (120 vs 40 ns/matmul).

All x DMAs are issued before the W DMAs on both HWDGE rings, so the
fc matmuls (which need all four samples' feats) hide under the W
stream instead of trailing the x stream.
"""

import sys

import numpy as np
import ml_dtypes

for _p in ("/opt/trn_rl_repo",):
    if _p not in sys.path:
        sys.path.append(_p)

# Problem shapes (hardcoded per contract).
B, N, D, OUT = 32, 20000, 64, 256
NCORES = 8
BPC = B // NCORES            # samples per core
NPAIR = BPC // 2             # sample pairs per core
P = 128                      # SBUF partitions / matmul contraction tile
NCH = (N + P - 1) // P       # 157 contraction chunks of 128 rows
NPAD = NCH * P               # 20096 rows after zero padding
FB = 2 * D + 1               # bytes per partition per chunk (pair + ones)
KC = (D * D) // P            # 32 fc contraction chunks
WSLICES = 8                  # W DMA slices (each covers 4 fc chunks)
# x DMA schedule per sample pair: (chunk offset, chunks). Last tile is
# small so the final G chunks finish right after the stream ends.
DMA_TILES = [(0, 28), (28, 28), (56, 28), (84, 28), (112, 28), (140, 17)]
FILL_PER_TILE = 1            # HAM-warming dummy matmuls per x tile

_CACHE = {}


def _split_drain_and_barrier(self, tick_clock, wait_clock):
    """Replacement for TileContext._drain_and_barrier emitting one drain per
    sem wait: this walrus vintage rejects >1 sync-wait per instruction."""
    import bass_rust
    import concourse.mybir as mybir

    drain_bi = self.nc.sync.drain()
    inst = drain_bi.ins
    wait_clock.add_sem_waits(
        drain_bi.ins, bass_rust.ScopedClock({None: tick_clock.global_clock})
    )
    waits = list(inst.sync_info.on_wait) if inst.sync_info else []
    if len(waits) > 1:
        # one pure sem-wait NoOp per extra wait (cheaper than extra drains)
        inst.sync_info = mybir.SyncInfo(on_wait=waits[:1], on_update=[])
        for w in waits[1:]:
            nop = mybir.InstNoOp(
                name=f"tailwait-{w.ant_name}",
                engine=mybir.EngineType.SP,
                sync_info=mybir.SyncInfo(on_wait=[w], on_update=[]),
                bass_nofuse=True,
            )
            self.nc.sync.add_instruction(nop)

    self.nc.all_engine_barrier()
    assert self.sems is not None
    popped = self.nc._tile_sem_poison_stack.pop()
    assert popped is self._sem_poison
    self.nc.clear_and_free_semaphores(list(self.sems.allocated().values()))
    self.nc.all_engine_barrier()


def _build_nc():
    import types

    import concourse.bass as bass
    import concourse.mybir as mybir
    from concourse.tile import TileContext

    dt = mybir.dt
    AF = mybir.ActivationFunctionType
    nc = bass.Bass()

    xin = nc.dram_tensor(
        "xin", [NPAIR, NCH * FB * P], dt.float8e4, kind="ExternalInput"
    )
    win = nc.dram_tensor("win", [P, KC * OUT], dt.float16, kind="ExternalInput")
    # cols 0:OUT: bias; cols OUT:OUT+BPC: ones (same row -- matmul
    # operands must start at partition 0/32/64)
    bin_ = nc.dram_tensor("bin", [1, OUT + BPC], dt.float32, kind="ExternalInput")
    yout = nc.dram_tensor("yout", [BPC, OUT], dt.float32, kind="ExternalOutput")

    # Walrus single-sync-wait discipline (see _split_drain_and_barrier):
    #  - x tiles get one pool slot per DMA (no slot reuse -> DMAs need 0
    #    waits), per-pair psum G tiles are not reused
    #  - the s columns are read/reshaped on DVE only; cross-engine joins
    #    funnel through DVE ticks so same-engine waits merge
    #  - PE "observes" each W slice's DMA lane via a dummy matmul right
    #    before the first fc matmul that reads the slice.
    tc = TileContext(nc)
    tc._drain_and_barrier = types.MethodType(_split_drain_and_barrier, tc)
    with tc:
        with (
            tc.tile_pool(name="const", bufs=1) as cpool,
            tc.tile_pool(name="xp", bufs=len(DMA_TILES) * NPAIR) as xpool,
            tc.tile_pool(name="small", bufs=2) as spool,
            tc.tile_pool(name="featp", bufs=1) as fpool,
            tc.tile_pool(name="gpsum", bufs=NPAIR, space="PSUM") as gpool,
            tc.tile_pool(name="rpsum", bufs=1, space="PSUM") as rpool,
            tc.tile_pool(name="opsum", bufs=1, space="PSUM") as opool,
        ):
            w_sb = cpool.tile([P, KC * OUT], dt.float16)
            bias_sb = cpool.tile([1, OUT + BPC], dt.float32)
            nc.scalar.dma_start(out=bias_sb[:], in_=bin_[:])

            # The two HWDGE rings drain strictly in issue order per ring.
            ring = [nc.sync, nc.scalar]
            rr = [0]

            def ring_dma(out, in_, force=None):
                r = force if force is not None else rr[0] % 2
                if force is None:
                    rr[0] += 1
                ring[r].dma_start(out=out, in_=in_)

            # feat_sb[p, c, bb] = flattened cov for sample bb, fc-chunk
            # layout: element k = c*128 + p of cov.flatten(); chunk c stacks
            # cov[:, 2c] on partitions 0:64 and cov[:, 2c+1] on 64:128.
            feat_sb = fpool.tile([P, KC, BPC], dt.float16)

            po = opool.tile([BPC, OUT], dt.float32)
            pdum = opool.tile([1, 512], dt.float32, tag="pdum")

            # One psum bank holds all four samples' rps regions at disjoint
            # column offsets; each region is only touched by its own chain,
            # so interleaved start=True zero-marking never clobbers live
            # data.
            rq = rpool.tile([D, BPC * D], dt.float32, tag="rq")

            # Scratch for the s-column -> s-row transpose. Only column 0 is
            # ever written; the 32x32 block transpose routes in-column j to
            # out-row j, so the junk in columns 1:32 lands only on output
            # rows we never read (everything but rows 0 and 32).
            s32 = cpool.tile([D, 32], dt.float32, tag="s32")
            # rsb[64q:64q+64] holds R = s s^T/(N(N-1)) for pair-member q;
            # the base partition matches ge's so the fused feat ops see all
            # inputs at one partition offset.
            rsb = cpool.tile([P, D], dt.float32, tag="rsb")

            # Pre-warm the PE clock gate (HAM) with dummy matmuls on a memset
            # tile while the first x tile is still in flight: the gate needs
            # ~3.4 us of sustained activity to lift the 1.2 GHz cold throttle.
            dumsrc = cpool.tile([P, 512], dt.float8e4)
            nc.vector.memset(dumsrc[:], 0.5)
            for _ in range(8):
                nc.tensor.matmul(
                    pdum[:], lhsT=dumsrc[:, 0:1], rhs=dumsrc[:, 0:512],
                    start=True, stop=True,
                )

            def do_pair(q):
                # pg[0:64, 0:64] = G of sample 2q, pg[64:128, 64:128] = G of
                # sample 2q+1, pg[64q':64q'+64, 128] = s of each. The
                # off-diagonal blocks are cross-sample junk (finite, unread).
                pg = gpool.tile([P, FB], dt.float32, tag="pg")
                xts = []
                for ti, (i0, nblk) in enumerate(DMA_TILES):
                    xt = xpool.tile([P, nblk * FB], dt.float8e4, tag="xt")
                    xts.append(xt)
                    ring_dma(
                        xt[:],
                        xin[q, i0 * FB * P : (i0 + nblk) * FB * P].rearrange(
                            "(p f) -> p f", p=P
                        ),
                    )
                for ti, (i0, nblk) in enumerate(DMA_TILES):
                    xt = xts[ti]
                    for j in range(nblk):
                        # 128-column stationary (both samples' x) triggers
                        # Fast Weight Load; the moving operand adds the ones
                        # column so column sums accumulate in psum col 128.
                        nc.tensor.matmul(
                            pg[:],
                            lhsT=xt[:, j * FB : j * FB + P],
                            rhs=xt[:, j * FB : (j + 1) * FB],
                            start=(i0 + j == 0),
                            stop=(i0 + j == NCH - 1),
                        )
                    # HAM-warming filler: keeps the PE activity monitor from
                    # re-throttling the clock during DMA stalls.
                    for _ in range(FILL_PER_TILE):
                        nc.tensor.matmul(
                            pdum[:, 0:256], lhsT=xt[:, 0:1], rhs=xt[:, 0:256],
                            start=True, stop=True,
                        )
                for qq in range(2):
                    bb = 2 * q + qq
                    base = D * qq
                    # s column -> row: copy into col 0 of s32, 32x32 block
                    # transpose, stitch the two 32-halves (rows 0 and 32).
                    nc.vector.tensor_copy(
                        s32[:, 0:1], pg[base : base + D, 2 * D : FB]
                    )
                    sT = spool.tile([D, 32], dt.float32, tag="sT")
                    nc.vector.transpose(sT[:], s32[:])
                    s_pos = spool.tile([1, D], dt.float32, tag="spos")
                    nc.vector.tensor_copy(s_pos[0:1, 0:32], sT[0:1, 0:32])
                    nc.vector.tensor_copy(s_pos[0:1, 32:D], sT[32:33, 0:32])
                    s_scl = spool.tile([1, D], dt.float32, tag="sscl")
                    nc.vector.tensor_scalar_mul(
                        s_scl[:], s_pos[:], 1.0 / (N * (N - 1.0))
                    )
                    rps = rq[:, bb * D : (bb + 1) * D]
                    nc.tensor.matmul(
                        rps, lhsT=s_scl[:], rhs=s_pos[:], start=True,
                        stop=True, skip_group_check=True,
                    )
                    nc.vector.tensor_copy(rsb[base : base + D, :], rps)
                    ge = pg[base : base + D, base : base + D].rearrange(
                        "p (c two) -> p c two", two=2
                    )
                    re = rsb[base : base + D, :].rearrange(
                        "p (c two) -> p c two", two=2
                    )
                    # feat = G/(N-1) - s s^T/(N(N-1))  (= cov), cast to fp16
                    nc.vector.scalar_tensor_tensor(
                        feat_sb[0:D, :, bb], ge[:, :, 0], 1.0 / (N - 1.0),
                        re[:, :, 0], op0=mybir.AluOpType.mult,
                        op1=mybir.AluOpType.subtract,
                    )
                    nc.vector.scalar_tensor_tensor(
                        feat_sb[D:P, :, bb], ge[:, :, 1], 1.0 / (N - 1.0),
                        re[:, :, 1], op0=mybir.AluOpType.mult,
                        op1=mybir.AluOpType.subtract,
                    )
                # keep the PE array warm across the pair-boundary stall
                for _ in range(0 if q == 0 else 2):
                    nc.tensor.matmul(
                        pdum[:, 0:256], lhsT=xts[-1][:, 0:1],
                        rhs=xts[-1][:, 0:256], start=True, stop=True,
                    )

            for q in range(NPAIR):
                do_pair(q)

            # W rides both rings AFTER the entire x stream: the fc matmuls
            # pace behind the arriving W slices and hide under their DMA
            # time instead of trailing the x stream.
            WSL = KC * OUT // WSLICES
            for c in range(WSLICES):
                ring_dma(
                    w_sb[:, c * WSL : (c + 1) * WSL],
                    win[:, c * WSL : (c + 1) * WSL],
                    force=c % 2,
                )

            # Open the fc accumulation with the bias row: po = 1 * bias'.
            nc.tensor.matmul(
                po[:], lhsT=bias_sb[0:1, OUT : OUT + BPC], rhs=bias_sb[0:1, 0:OUT],
                start=True, stop=False,
            )
            # fc: out[bb, o] = bias'[o] + sum_k feat[k, bb] * W[k, o].
            # Before the first chunk of each W slice, a 1x1 dummy matmul
            # observes that slice's DMA lane so the fc matmul itself only
            # needs its feat (DVE) wait.
            CPS = KC // WSLICES
            for c in range(KC):
                if c % CPS == 0:
                    sl = c // CPS
                    nc.tensor.matmul(
                        pdum[0:1, 0:1],
                        lhsT=w_sb[0:1, sl * WSL : sl * WSL + 1],
                        rhs=w_sb[0:1, sl * WSL : sl * WSL + 1],
                        start=True, stop=True,
                    )
                nc.tensor.matmul(
                    po[:],
                    lhsT=feat_sb[:, c, :],
                    rhs=w_sb[:, c * OUT : (c + 1) * OUT],
                    start=False,
                    stop=(c == KC - 1),
                )

            # L2 normalize rows: out = po / sqrt(sum(po^2)). ACT Square with
            # row-sum accumulator (square and sqrt share one ACT table set),
            # ACT sqrt, DVE reciprocal, one DVE scale.
            sq = spool.tile([BPC, OUT], dt.float32, tag="sq")
            ss = spool.tile([BPC, 1], dt.float32, tag="ss")
            nc.scalar.activation(sq[:], po[:], AF.Square, accum_out=ss[:])
            nrm = spool.tile([BPC, 1], dt.float32, tag="nrm")
            nc.scalar.activation(nrm[:], ss[:], AF.Sqrt)
            inv = spool.tile([BPC, 1], dt.float32, tag="inv")
            nc.vector.reciprocal(inv[:], nrm[:])
            out_sb = spool.tile([BPC, OUT], dt.float32, tag="osb")
            nc.vector.tensor_scalar_mul(out_sb[:], po[:], inv[:])
            # SWDGE: an HWDGE yout DMA would need a DMAHW lane-reuse wait on
            # top of its DVE data wait (2 waits > walrus limit).
            nc.gpsimd.dma_start(out=yout[:], in_=out_sb[:])

    return nc


def _get_nc():
    if "nc" not in _CACHE:
        _CACHE["nc"] = _build_nc()
    return _CACHE["nc"]


def _pack_inputs(x, W, b):
    x = np.asarray(x, dtype=np.float32)
    W = np.asarray(W, dtype=np.float32)
    b = np.asarray(b, dtype=np.float32)

    xpad = np.zeros((B, NPAD, D), dtype=ml_dtypes.float8_e4m3)
    xpad[:, :N, :] = x.astype(ml_dtypes.float8_e4m3)
    # Pair samples (2q, 2q+1); chunk i, partition p holds row i*128+p of
    # both samples plus a shared ones byte: [x_a(64) | x_b(64) | 1].
    # [B,NPAD,D] -> [B/2, 2, NCH, P, D] -> [B/2, P, NCH, 2, D]
    xq = xpad.reshape(B // 2, 2, NCH, P, D).transpose(0, 3, 2, 1, 4)
    xq = xq.reshape(B // 2, P, NCH, 2 * D)
    ones = np.ones((B // 2, P, NCH, 1), dtype=ml_dtypes.float8_e4m3)
    augT = np.concatenate([xq, ones], axis=3).reshape(B // 2, P, NCH * FB)
    # regroup into DMA tiles: each dma_start reads one contiguous extent
    parts = []
    for (i0, nblk) in DMA_TILES:
        blk = augT[:, :, i0 * FB : (i0 + nblk) * FB]
        parts.append(blk.reshape(B // 2, P * nblk * FB))
    augT = np.ascontiguousarray(np.concatenate(parts, axis=1))

    wp = np.ascontiguousarray(
        W.reshape(KC, P, OUT).transpose(1, 0, 2)
    ).reshape(P, KC * OUT).astype(np.float16)
    bp = np.concatenate([b, np.ones(BPC, np.float32)]).reshape(1, OUT + BPC)

    return [
        {
            "xin": np.ascontiguousarray(augT[c * NPAIR : (c + 1) * NPAIR]),
            "win": wp,
            "bin": bp,
        }
        for c in range(NCORES)
    ]


def run(x, W, b, trace=False):
    from concourse.bass_utils import run_bass_kernel_spmd

    nc = _get_nc()
    in_maps = _pack_inputs(x, W, b)
    res = run_bass_kernel_spmd(nc, in_maps, list(range(NCORES)), trace=trace)
    out = np.concatenate(
        [res.results[c]["yout"] for c in range(NCORES)], axis=0
    ).astype(np.float32)
    return out, res


def kernel(x, W, b):
    out, _ = run(x, W, b, trace=False)
    return out


# revision 21
# speedup vs baseline: 1.5977x; 1.0496x over previous
"""Trainium2 Bass kernel for per-sample covariance pooling + fc + L2 norm.

Reference computation (per sample b of B=32):
    xc  = x[b] - mean(x[b], axis=0)            # x[b]: [N=20000, D=64]
    cov = xc.T @ xc / (N-1)                    # [64, 64]
    out = normalize(cov.flatten() @ W + b)     # [256]

Kernel formulation (scale/norm invariant):
    G = x.T @ x, s = sum(x, axis=0)            # one PE pass over x
    cov = (G - s s^T / N) / (N-1)
    out = normalize(cov.flatten() @ W + b)

Sharding: data-parallel over batch, 4 samples per core on 8 cores; W
and bias replicated. x is host-packed to fp8 e4m3 (end-to-end rel err
~2.3e-3 vs the 2e-2 gate). Two samples ride side by side per
partition row: chunk layout [x_a(64) | x_b(64) | ones(1)], so the
Gram matmul has a 128-column stationary operand -- exactly the shape
that triggers the compiler's Fast Weight Load (4 fp8/cycle; the
dominant LDWEIGHTS cost of tall-skinny Grams drops 4x) -- and one
matmul per 128 rows yields both samples' G blocks plus both column
sums (from the ones column) in a [128, 129] psum. DoubleRow mode is
deliberately NOT used: at free-dim 64 it disables FWL and measures
~3x slower (120 vs 40 ns/matmul).

All x DMAs are issued before the W DMAs on both HWDGE rings, so the
fc matmuls (which need all four samples' feats) hide under the W
stream instead of trailing the x stream.
"""

import sys

import numpy as np
import ml_dtypes

for _p in ("/opt/trn_rl_repo",):
    if _p not in sys.path:
        sys.path.append(_p)

# Problem shapes (hardcoded per contract).
B, N, D, OUT = 32, 20000, 64, 256
NCORES = 8
BPC = B // NCORES            # samples per core
NPAIR = BPC // 2             # sample pairs per core
P = 128                      # SBUF partitions / matmul contraction tile
NCH = (N + P - 1) // P       # 157 contraction chunks of 128 rows
NPAD = NCH * P               # 20096 rows after zero padding
FB = 2 * D + 1               # bytes per partition per chunk (pair + ones)
KC = (D * D) // P            # 32 fc contraction chunks
WSLICES = 8                  # W DMA slices (each covers 4 fc chunks)
# x DMA schedule per sample pair: (chunk offset, chunks). Last tile is
# small so the final G chunks finish right after the stream ends.
DMA_TILES = [(0, 28), (28, 28), (56, 28), (84, 28), (112, 28), (140, 17)]
FILL_PER_TILE = 1            # HAM-warming dummy matmuls per x tile

_CACHE = {}


def _split_drain_and_barrier(self, tick_clock, wait_clock):
    """Replacement for TileContext._drain_and_barrier emitting one drain per
    sem wait: this walrus vintage rejects >1 sync-wait per instruction."""
    import bass_rust
    import concourse.mybir as mybir

    drain_bi = self.nc.sync.drain()
    inst = drain_bi.ins
    wait_clock.add_sem_waits(
        drain_bi.ins, bass_rust.ScopedClock({None: tick_clock.global_clock})
    )
    waits = list(inst.sync_info.on_wait) if inst.sync_info else []
    if len(waits) > 1:
        # one pure sem-wait NoOp per extra wait (cheaper than extra drains)
        inst.sync_info = mybir.SyncInfo(on_wait=waits[:1], on_update=[])
        for w in waits[1:]:
            nop = mybir.InstNoOp(
                name=f"tailwait-{w.ant_name}",
                engine=mybir.EngineType.SP,
                sync_info=mybir.SyncInfo(on_wait=[w], on_update=[]),
                bass_nofuse=True,
            )
            self.nc.sync.add_instruction(nop)

    self.nc.all_engine_barrier()
    assert self.sems is not None
    popped = self.nc._tile_sem_poison_stack.pop()
    assert popped is self._sem_poison
    self.nc.clear_and_free_semaphores(list(self.sems.allocated().values()))
    self.nc.all_engine_barrier()


def _build_nc():
    import types

    import concourse.bass as bass
    import concourse.mybir as mybir
    from concourse.tile import TileContext

    dt = mybir.dt
    AF = mybir.ActivationFunctionType
    nc = bass.Bass()

    xin = nc.dram_tensor(
        "xin", [NPAIR, NCH * FB * P], dt.float8e4, kind="ExternalInput"
    )
    win = nc.dram_tensor("win", [P, KC * OUT], dt.float16, kind="ExternalInput")
    # cols 0:OUT: bias; cols OUT:OUT+BPC: ones (same row -- matmul
    # operands must start at partition 0/32/64)
    bin_ = nc.dram_tensor("bin", [1, OUT + BPC], dt.float32, kind="ExternalInput")
    yout = nc.dram_tensor("yout", [BPC, OUT], dt.float32, kind="ExternalOutput")

    # Walrus single-sync-wait discipline (see _split_drain_and_barrier):
    #  - x tiles get one pool slot per DMA (no slot reuse -> DMAs need 0
    #    waits), per-pair psum G tiles are not reused
    #  - the s columns are read/reshaped on DVE only; cross-engine joins
    #    funnel through DVE ticks so same-engine waits merge
    #  - PE "observes" each W slice's DMA lane via a dummy matmul right
    #    before the first fc matmul that reads the slice.
    tc = TileContext(nc)
    tc._drain_and_barrier = types.MethodType(_split_drain_and_barrier, tc)
    with tc:
        with (
            tc.tile_pool(name="const", bufs=1) as cpool,
            tc.tile_pool(name="xp", bufs=len(DMA_TILES) * NPAIR) as xpool,
            tc.tile_pool(name="small", bufs=2) as spool,
            tc.tile_pool(name="featp", bufs=1) as fpool,
            tc.tile_pool(name="gpsum", bufs=NPAIR, space="PSUM") as gpool,
            tc.tile_pool(name="rpsum", bufs=1, space="PSUM") as rpool,
            tc.tile_pool(name="opsum", bufs=1, space="PSUM") as opool,
        ):
            w_sb = cpool.tile([P, KC * OUT], dt.float16)
            bias_sb = cpool.tile([1, OUT + BPC], dt.float32)
            nc.scalar.dma_start(out=bias_sb[:], in_=bin_[:])

            # The two HWDGE rings drain strictly in issue order per ring.
            ring = [nc.sync, nc.scalar]
            rr = [0]

            def ring_dma(out, in_, force=None):
                r = force if force is not None else rr[0] % 2
                if force is None:
                    rr[0] += 1
                ring[r].dma_start(out=out, in_=in_)

            # feat_sb[p, c, bb] = flattened cov for sample bb, fc-chunk
            # layout: element k = c*128 + p of cov.flatten(); chunk c stacks
            # cov[:, 2c] on partitions 0:64 and cov[:, 2c+1] on 64:128.
            feat_sb = fpool.tile([P, KC, BPC], dt.float16)

            po = opool.tile([BPC, OUT], dt.float32)
            pdum = opool.tile([1, 512], dt.float32, tag="pdum")

            # One psum bank holds all four samples' rps regions at disjoint
            # column offsets; each region is only touched by its own chain,
            # so interleaved start=True zero-marking never clobbers live
            # data.
            rq = rpool.tile([D, BPC * D], dt.float32, tag="rq")

            # Scratch for the s-column -> s-row transpose. Only column 0 is
            # ever written; the 32x32 block transpose routes in-column j to
            # out-row j, so the junk in columns 1:32 lands only on output
            # rows we never read (everything but rows 0 and 32).
            s32 = cpool.tile([D, 32], dt.float32, tag="s32")
            # rsb[64q:64q+64] holds R = s s^T/(N(N-1)) for pair-member q;
            # the base partition matches ge's so the fused feat ops see all
            # inputs at one partition offset.
            rsb = cpool.tile([P, D], dt.float32, tag="rsb")

            # Pre-warm the PE clock gate (HAM) with dummy matmuls on a memset
            # tile while the first x tile is still in flight: the gate needs
            # ~3.4 us of sustained activity to lift the 1.2 GHz cold throttle.
            dumsrc = cpool.tile([P, 512], dt.float8e4)
            nc.vector.memset(dumsrc[:], 0.5)
            for _ in range(8):
                nc.tensor.matmul(
                    pdum[:], lhsT=dumsrc[:, 0:1], rhs=dumsrc[:, 0:512],
                    start=True, stop=True,
                )

            def do_pair(q):
                # pg[0:64, 0:64] = G of sample 2q, pg[64:128, 64:128] = G of
                # sample 2q+1, pg[64q':64q'+64, 128] = s of each. The
                # off-diagonal blocks are cross-sample junk (finite, unread).
                pg = gpool.tile([P, FB], dt.float32, tag="pg")
                xts = []
                # Ring0's sequencer reaches its first dma_start ~4 us before
                # ring1's (ring1 is behind the bias DMA and ACT preamble), so
                # pair 0's leading tiles all go to ring0 and ring1 carries
                # the tails -- this keeps tile arrival ahead of the PE's
                # ~2 us/tile consumption with no mid-stream stalls.
                RINGS = ([0, 0, 0, 0, 1, 1], [0, 0, 1, 1, 1, 1])[q]
                for ti, (i0, nblk) in enumerate(DMA_TILES):
                    xt = xpool.tile([P, nblk * FB], dt.float8e4, tag="xt")
                    xts.append(xt)
                    ring_dma(
                        xt[:],
                        xin[q, i0 * FB * P : (i0 + nblk) * FB * P].rearrange(
                            "(p f) -> p f", p=P
                        ),
                        force=RINGS[ti],
                    )
                for ti, (i0, nblk) in enumerate(DMA_TILES):
                    xt = xts[ti]
                    for j in range(nblk):
                        # 128-column stationary (both samples' x) triggers
                        # Fast Weight Load; the moving operand adds the ones
                        # column so column sums accumulate in psum col 128.
                        nc.tensor.matmul(
                            pg[:],
                            lhsT=xt[:, j * FB : j * FB + P],
                            rhs=xt[:, j * FB : (j + 1) * FB],
                            start=(i0 + j == 0),
                            stop=(i0 + j == NCH - 1),
                        )
                    # HAM-warming filler: keeps the PE activity monitor from
                    # re-throttling the clock during DMA stalls.
                    for _ in range(FILL_PER_TILE):
                        nc.tensor.matmul(
                            pdum[:, 0:256], lhsT=xt[:, 0:1], rhs=xt[:, 0:256],
                            start=True, stop=True,
                        )
                for qq in range(2):
                    bb = 2 * q + qq
                    base = D * qq
                    # s column -> row: copy into col 0 of s32, 32x32 block
                    # transpose, stitch the two 32-halves (rows 0 and 32).
                    nc.vector.tensor_copy(
                        s32[:, 0:1], pg[base : base + D, 2 * D : FB]
                    )
                    sT = spool.tile([D, 32], dt.float32, tag="sT")
                    nc.vector.transpose(sT[:], s32[:])
                    s_pos = spool.tile([1, D], dt.float32, tag="spos")
                    nc.vector.tensor_copy(s_pos[0:1, 0:32], sT[0:1, 0:32])
                    nc.vector.tensor_copy(s_pos[0:1, 32:D], sT[32:33, 0:32])
                    s_scl = spool.tile([1, D], dt.float32, tag="sscl")
                    nc.vector.tensor_scalar_mul(
                        s_scl[:], s_pos[:], 1.0 / (N * (N - 1.0))
                    )
                    rps = rq[:, bb * D : (bb + 1) * D]
                    nc.tensor.matmul(
                        rps, lhsT=s_scl[:], rhs=s_pos[:], start=True,
                        stop=True, skip_group_check=True,
                    )
                    nc.vector.tensor_copy(rsb[base : base + D, :], rps)
                    ge = pg[base : base + D, base : base + D].rearrange(
                        "p (c two) -> p c two", two=2
                    )
                    re = rsb[base : base + D, :].rearrange(
                        "p (c two) -> p c two", two=2
                    )
                    # feat = G/(N-1) - s s^T/(N(N-1))  (= cov), cast to fp16
                    nc.vector.scalar_tensor_tensor(
                        feat_sb[0:D, :, bb], ge[:, :, 0], 1.0 / (N - 1.0),
                        re[:, :, 0], op0=mybir.AluOpType.mult,
                        op1=mybir.AluOpType.subtract,
                    )
                    nc.vector.scalar_tensor_tensor(
                        feat_sb[D:P, :, bb], ge[:, :, 1], 1.0 / (N - 1.0),
                        re[:, :, 1], op0=mybir.AluOpType.mult,
                        op1=mybir.AluOpType.subtract,
                    )
                # keep the PE array warm across the pair-boundary stall
                for _ in range(0 if q == 0 else 2):
                    nc.tensor.matmul(
                        pdum[:, 0:256], lhsT=xts[-1][:, 0:1],
                        rhs=xts[-1][:, 0:256], start=True, stop=True,
                    )

            for q in range(NPAIR):
                do_pair(q)

            # W rides both rings AFTER the entire x stream: the fc matmuls
            # pace behind the arriving W slices and hide under their DMA
            # time instead of trailing the x stream.
            WSL = KC * OUT // WSLICES
            WRINGS = [0, 1, 0, 1, 0, 1, 1, 1]  # rebalance bytes across rings
            for c in range(WSLICES):
                ring_dma(
                    w_sb[:, c * WSL : (c + 1) * WSL],
                    win[:, c * WSL : (c + 1) * WSL],
                    force=WRINGS[c],
                )

            # Bridge the PE idle gap while the last pair's feat chain runs on
            # DVE: without activity the HAM re-throttles the clock to 1.2 GHz
            # and the fc then runs at half speed.
            for _ in range(10):
                nc.tensor.matmul(
                    pdum[:, 0:256], lhsT=dumsrc[:, 0:1], rhs=dumsrc[:, 0:256],
                    start=True, stop=True,
                )
            # Open the fc accumulation with the bias row: po = 1 * bias'.
            nc.tensor.matmul(
                po[:], lhsT=bias_sb[0:1, OUT : OUT + BPC], rhs=bias_sb[0:1, 0:OUT],
                start=True, stop=False,
            )
            # fc: out[bb, o] = bias'[o] + sum_k feat[k, bb] * W[k, o].
            # Before the first chunk of each W slice, a 1x1 dummy matmul
            # observes that slice's DMA lane so the fc matmul itself only
            # needs its feat (DVE) wait.
            CPS = KC // WSLICES
            for c in range(KC):
                if c % CPS == 0:
                    sl = c // CPS
                    nc.tensor.matmul(
                        pdum[0:1, 0:1],
                        lhsT=w_sb[0:1, sl * WSL : sl * WSL + 1],
                        rhs=w_sb[0:1, sl * WSL : sl * WSL + 1],
                        start=True, stop=True,
                    )
                nc.tensor.matmul(
                    po[:],
                    lhsT=feat_sb[:, c, :],
                    rhs=w_sb[:, c * OUT : (c + 1) * OUT],
                    start=False,
                    stop=(c == KC - 1),
                )

            # L2 normalize rows: out = po / sqrt(sum(po^2)). ACT Square with
            # row-sum accumulator (square and sqrt share one ACT table set),
            # ACT sqrt, DVE reciprocal, one DVE scale.
            sq = spool.tile([BPC, OUT], dt.float32, tag="sq")
            ss = spool.tile([BPC, 1], dt.float32, tag="ss")
            nc.scalar.activation(sq[:], po[:], AF.Square, accum_out=ss[:])
            nrm = spool.tile([BPC, 1], dt.float32, tag="nrm")
            nc.scalar.activation(nrm[:], ss[:], AF.Sqrt)
            inv = spool.tile([BPC, 1], dt.float32, tag="inv")
            nc.vector.reciprocal(inv[:], nrm[:])
            out_sb = spool.tile([BPC, OUT], dt.float32, tag="osb")
            nc.vector.tensor_scalar_mul(out_sb[:], po[:], inv[:])
            # SWDGE: an HWDGE yout DMA would need a DMAHW lane-reuse wait on
            # top of its DVE data wait (2 waits > walrus limit).
            nc.gpsimd.dma_start(out=yout[:], in_=out_sb[:])

    return nc


def _get_nc():
    if "nc" not in _CACHE:
        _CACHE["nc"] = _build_nc()
    return _CACHE["nc"]


def _pack_inputs(x, W, b):
    x = np.asarray(x, dtype=np.float32)
    W = np.asarray(W, dtype=np.float32)
    b = np.asarray(b, dtype=np.float32)

    xpad = np.zeros((B, NPAD, D), dtype=ml_dtypes.float8_e4m3)
    xpad[:, :N, :] = x.astype(ml_dtypes.float8_e4m3)
    # Pair samples (2q, 2q+1); chunk i, partition p holds row i*128+p of
    # both samples plus a shared ones byte: [x_a(64) | x_b(64) | 1].
    # [B,NPAD,D] -> [B/2, 2, NCH, P, D] -> [B/2, P, NCH, 2, D]
    xq = xpad.reshape(B // 2, 2, NCH, P, D).transpose(0, 3, 2, 1, 4)
    xq = xq.reshape(B // 2, P, NCH, 2 * D)
    ones = np.ones((B // 2, P, NCH, 1), dtype=ml_dtypes.float8_e4m3)
    augT = np.concatenate([xq, ones], axis=3).reshape(B // 2, P, NCH * FB)
    # regroup into DMA tiles: each dma_start reads one contiguous extent
    parts = []
    for (i0, nblk) in DMA_TILES:
        blk = augT[:, :, i0 * FB : (i0 + nblk) * FB]
        parts.append(blk.reshape(B // 2, P * nblk * FB))
    augT = np.ascontiguousarray(np.concatenate(parts, axis=1))

    wp = np.ascontiguousarray(
        W.reshape(KC, P, OUT).transpose(1, 0, 2)
    ).reshape(P, KC * OUT).astype(np.float16)
    bp = np.concatenate([b, np.ones(BPC, np.float32)]).reshape(1, OUT + BPC)

    return [
        {
            "xin": np.ascontiguousarray(augT[c * NPAIR : (c + 1) * NPAIR]),
            "win": wp,
            "bin": bp,
        }
        for c in range(NCORES)
    ]


def run(x, W, b, trace=False):
    from concourse.bass_utils import run_bass_kernel_spmd

    nc = _get_nc()
    in_maps = _pack_inputs(x, W, b)
    res = run_bass_kernel_spmd(nc, in_maps, list(range(NCORES)), trace=trace)
    out = np.concatenate(
        [res.results[c]["yout"] for c in range(NCORES)], axis=0
    ).astype(np.float32)
    return out, res


def kernel(x, W, b):
    out, _ = run(x, W, b, trace=False)
    return out
